# revision 17
# baseline (speedup 1.0000x reference)
"""Trainium2 Bass kernel for nn_DistLayer (GNN message passing layer).

Computes, for full inputs (see reference):
    pa = relu(seg_mean(x[:, :128], atom_idx, 1024))[atom_idx]
    pe = relu(seg_mean(x[:, 128:], ele_idx, 100))[ele_idx]
    h  = concat([dist_feat, pa, pe], 1) @ W1 (+ b1)
    out = relu(batchnorm_train(h; gamma, beta) + x)

Note b1 provably cancels in (h - mean(h)), so it is ignored.

Strategy (8 cores, data-parallel over rows):
  - Rows sharded 25000/core; each shard bucketed by atom_idx>>7 into 8
    fixed-size 3456-row windows (pad rows are inert), so segment-sums
    use narrow [128,128] one-hot matmuls accumulated in PSUM per window.
  - AllReduce #1 combines per-core segment sums [128, 1152].
  - Pooled means -> relu -> matmul with W1 halves to build per-segment
    contribution tables; rows are fetched back with dma_gather (bf16).
  - h kept resident in SBUF (bf16); BN stats via ones-matmul column sums,
    AllReduce #2 on [1,512], then fused affine+residual+relu output pass.
"""
import sys

sys.path.insert(0, "/opt/trn_rl_repo")

import numpy as np

import concourse.bass as bass
import concourse.mybir as mybir
import concourse.tile as tile
from concourse import bacc
from concourse.bass_utils import run_bass_kernel_spmd, axon_active

# problem constants
N = 200000
NAE = 128
NDE = 128
G = 1024
E = 100
NCORES = 8
RPC = N // NCORES          # 25000 rows per core
NW = 8                     # windows (atom segment buckets of 128)
CPW = 27                   # chunks (of 128 rows) per window
BUCKET = CPW * 128         # 3456 padded rows per window
TROWS = NW * BUCKET        # 27648 padded rows per core
T = TROWS // 128           # 216 chunks
GG = 8                     # chunks per dma_gather group (1024 idx; 1536 hangs HW)
SUMW = G + 128             # 1152: [atom sums | ele sums(padded to 128)]
ZROW_A = G                 # zero row index in atom table
ZROW_E = E                 # zero row index in ele table
EPS = 1e-5
INV_N = 1.0 / N

F32 = mybir.dt.float32
BF16 = mybir.dt.bfloat16
I16 = mybir.dt.int16

_CACHED_PROGRAM = None


class Cfg:
    """Size configuration; defaults = the real problem."""

    def __init__(self, rpc=RPC, cpw=CPW, gg=GG, debug=None):
        self.rpc = rpc
        self.cpw = cpw
        self.bucket = cpw * 128
        self.trows = NW * self.bucket
        self.t = self.trows // 128
        self.gg = gg
        self.inv_n = 1.0 / (rpc * NCORES)
        self.debug = debug


def _build_program(cfg=None):
    cfg = cfg or Cfg()
    CPW, GG, TROWS, T = cfg.cpw, cfg.gg, cfg.trows, cfg.t
    INV_N = cfg.inv_n
    dbg = (not axon_active()) if cfg.debug is None else cfg.debug
    nc = bacc.Bacc(
        "TRN2",
        target_bir_lowering=False,
        debug=dbg,
        num_devices=NCORES,
    )

    # per-core external I/O
    xs = nc.dram_tensor("xs", [TROWS, 2 * NAE], F32, kind="ExternalInput")
    dsT = nc.dram_tensor("dsT", [NDE, TROWS], F32, kind="ExternalInput")
    aw = nc.dram_tensor("aw", [TROWS, 1], F32, kind="ExternalInput")
    ew = nc.dram_tensor("ew", [TROWS, 1], F32, kind="ExternalInput")
    ag16 = nc.dram_tensor("ag16", [128, TROWS // 16], I16, kind="ExternalInput")
    eg16 = nc.dram_tensor("eg16", [128, TROWS // 16], I16, kind="ExternalInput")
    w1 = nc.dram_tensor("w1", [3 * 128, 2 * NAE], F32, kind="ExternalInput")
    gb = nc.dram_tensor("gb", [1, 512], F32, kind="ExternalInput")
    rcb = nc.dram_tensor("rcb", [128, SUMW], F32, kind="ExternalInput")
    arange = nc.dram_tensor("arange", [128, 128], F32, kind="ExternalInput")
    ones1 = nc.dram_tensor("ones1", [1, 128], F32, kind="ExternalInput")
    out_d = nc.dram_tensor("out", [TROWS, 2 * NAE], F32, kind="ExternalOutput")

    # internal DRAM (collective bounce buffers + tables)
    cc1_in = nc.dram_tensor("cc1_in", [128, SUMW], F32)
    cc1_out = nc.dram_tensor("cc1_out", [128, SUMW], F32, addr_space="Shared")
    cc2_in = nc.dram_tensor("cc2_in", [1, 512], F32)
    cc2_out = nc.dram_tensor("cc2_out", [1, 512], F32, addr_space="Shared")
    table_a = nc.dram_tensor("table_a", [G + 128, 2 * NAE], BF16)
    table_e = nc.dram_tensor("table_e", [128, 2 * NAE], BF16)

    with tile.TileContext(nc) as tc:
        with (
            tc.tile_pool(name="const", bufs=1) as cp,
            tc.tile_pool(name="hcache", bufs=1) as hp,
            tc.tile_pool(name="xload", bufs=3) as xp,
            tc.tile_pool(name="small", bufs=4) as sp,
            tc.tile_pool(name="work", bufs=2) as wp,
            tc.tile_pool(name="gather", bufs=2) as gp,
            tc.tile_pool(name="outp", bufs=3) as op_,
        ):
            # ---- constants into SBUF
            w1_parts = []
            for i in range(3):
                tf = wp.tile([128, 256], F32, tag="w1f")
                nc.sync.dma_start(tf[:], w1[i * 128 : (i + 1) * 128, :])
                tb = cp.tile([128, 256], BF16, tag=f"w1b{i}")
                nc.scalar.copy(tb[:], tf[:])
                w1_parts.append(tb)
            w1d, w1a, w1e = w1_parts

            ar_sb = cp.tile([128, 128], F32, tag="arange")
            nc.sync.dma_start(ar_sb[:], arange[:])
            rcb_sb = cp.tile([128, SUMW], F32, tag="rcb")
            nc.sync.dma_start(rcb_sb[:], rcb[:])
            ones1_sb = cp.tile([1, 128], F32, tag="ones1")
            nc.sync.dma_start(ones1_sb[:], ones1[:])
            gb_sb = cp.tile([1, 512], F32, tag="gb")
            nc.sync.dma_start(gb_sb[:], gb[:])
            ag_sb = cp.tile([128, TROWS // 16], I16, tag="ag")
            nc.sync.dma_start(ag_sb[:], ag16[:])
            eg_sb = cp.tile([128, TROWS // 16], I16, tag="eg")
            nc.sync.dma_start(eg_sb[:], eg16[:])
            onescol = cp.tile([128, 1], BF16, tag="onescol")
            nc.vector.memset(onescol[:], 1.0)

            # ---- Stage A: local segment sums (transposed: [ae_dim, seg])
            acc = cp.tile([128, SUMW], F32, tag="acc")
            nc.vector.memset(acc[:], 0.0)

            psA = tc.alloc_tile_pool(name="psA", bufs=2, space="PSUM")
            for w in range(NW):
                ps_a = psA.tile([128, 128], F32, tag="ps_a")
                ps_e = psA.tile([128, 128], F32, tag="ps_e")
                for j in range(CPW):
                    t = w * CPW + j
                    xt = xp.tile([128, 256], F32, tag="xt")
                    nc.sync.dma_start(xt[:], xs[t * 128 : (t + 1) * 128, :])
                    xb = wp.tile([128, 256], BF16, tag="xb")
                    nc.scalar.copy(xb[:], xt[:])
                    ia = sp.tile([128, 1], F32, tag="ia")
                    nc.sync.dma_start(ia[:], aw[t * 128 : (t + 1) * 128, :])
                    ie = sp.tile([128, 1], F32, tag="ie")
                    nc.sync.dma_start(ie[:], ew[t * 128 : (t + 1) * 128, :])
                    oha = wp.tile([128, 128], BF16, tag="oha")
                    nc.vector.tensor_scalar(
                        out=oha[:], in0=ar_sb[:], scalar1=ia[:], scalar2=None,
                        op0=mybir.AluOpType.is_equal,
                    )
                    ohe = wp.tile([128, 128], BF16, tag="ohe")
                    nc.vector.tensor_scalar(
                        out=ohe[:], in0=ar_sb[:], scalar1=ie[:], scalar2=None,
                        op0=mybir.AluOpType.is_equal,
                    )
                    nc.tensor.matmul(
                        ps_a[:], lhsT=xb[:, 0:128], rhs=oha[:],
                        start=(j == 0), stop=(j == CPW - 1),
                    )
                    nc.tensor.matmul(
                        ps_e[:], lhsT=xb[:, 128:256], rhs=ohe[:],
                        start=(j == 0), stop=(j == CPW - 1),
                    )
                nc.vector.tensor_copy(acc[:, w * 128 : (w + 1) * 128], ps_a[:])
                nc.vector.tensor_add(acc[:, G : G + 128], acc[:, G : G + 128], ps_e[:])
            psA.release()
            psH = tc.alloc_tile_pool(name="psH", bufs=3, space="PSUM")
            psS = tc.alloc_tile_pool(name="psS", bufs=1, space="PSUM")

            # ---- AllReduce #1 (segment sums)
            nc.sync.dma_start(cc1_in[:], acc[:])
            nc.gpsimd.collective_compute(
                "AllReduce",
                mybir.AluOpType.add,
                replica_groups=[list(range(NCORES))],
                ins=[cc1_in[:]],
                outs=[cc1_out[:]],
            )
            # reuse `acc` for the global sums and means (in place)
            nc.sync.dma_start(acc[:], cc1_out[:])

            # ---- tables: relu(mean) @ W1 part, stored bf16 in DRAM
            nc.vector.tensor_mul(acc[:], acc[:], rcb_sb[:])
            rmeans = cp.tile([128, SUMW], BF16, tag="rmeans")
            nc.scalar.activation(rmeans[:], acc[:], mybir.ActivationFunctionType.Relu)

            for blk in range(NW):
                pst = psH.tile([128, 256], F32, tag="ps_h")
                nc.tensor.matmul(
                    pst[:], lhsT=rmeans[:, blk * 128 : (blk + 1) * 128], rhs=w1a[:],
                    start=True, stop=True,
                )
                tb = op_.tile([128, 256], BF16, tag="tbl")
                nc.vector.tensor_copy(tb[:], pst[:])
                nc.sync.dma_start(table_a[blk * 128 : (blk + 1) * 128, :], tb[:])
            zr = sp.tile([128, 256], BF16, tag="zr")
            nc.vector.memset(zr[:], 0.0)
            nc.sync.dma_start(table_a[G : G + 128, :], zr[:])

            pst = psH.tile([128, 256], F32, tag="ps_h")
            nc.tensor.matmul(
                pst[:], lhsT=rmeans[:, G : G + 128], rhs=w1e[:], start=True, stop=True
            )
            tb = op_.tile([128, 256], BF16, tag="tbl")
            nc.vector.tensor_copy(tb[:], pst[:])
            nc.sync.dma_start(table_e[:, :], tb[:])

            # ---- Stage C: h = dsT.T @ W1d + gather(table_a) + gather(table_e)
            hbuf = hp.tile([128, T, 256], BF16, tag="H")
            acc_s = cp.tile([1, 512], F32, tag="acc_s")
            nc.vector.memset(acc_s[:], 0.0)

            n_groups = (T + GG - 1) // GG
            for grp in range(n_groups):
                base = grp * GG
                gchunks = min(GG, T - base)
                nidx = gchunks * 128
                pa_buf = gp.tile([128, GG, 256], BF16, tag="pa")
                pe_buf = gp.tile([128, GG, 256], BF16, tag="pe")
                nc.gpsimd.dma_gather(
                    out_ap=pa_buf[:, 0:gchunks, :],
                    in_ap=table_a[:],
                    idxs_ap=ag_sb[:, base * 8 : base * 8 + nidx // 16],
                    num_idxs=nidx,
                    num_idxs_reg=nidx,
                    elem_size=256,
                )
                nc.gpsimd.dma_gather(
                    out_ap=pe_buf[:, 0:gchunks, :],
                    in_ap=table_e[:],
                    idxs_ap=eg_sb[:, base * 8 : base * 8 + nidx // 16],
                    num_idxs=nidx,
                    num_idxs_reg=nidx,
                    elem_size=256,
                )
                ps1 = psS.tile([1, 256], F32, tag="ps1")
                ps2 = psS.tile([1, 256], F32, tag="ps2")
                for j in range(gchunks):
                    t = base + j
                    dtf = xp.tile([128, 128], F32, tag="dtf")
                    nc.sync.dma_start(dtf[:], dsT[:, t * 128 : (t + 1) * 128])
                    dtb = wp.tile([128, 128], BF16, tag="dtb")
                    nc.vector.tensor_copy(dtb[:], dtf[:])
                    ps_h = psH.tile([128, 256], F32, tag="ps_h")
                    nc.tensor.matmul(ps_h[:], lhsT=dtb[:], rhs=w1d[:], start=True, stop=True)
                    t1 = wp.tile([128, 256], F32, tag="t1")
                    nc.vector.tensor_add(t1[:], pa_buf[:, j, :], pe_buf[:, j, :])
                    hs = hbuf[:, t, :]
                    nc.vector.tensor_add(hs, t1[:], ps_h[:])
                    hq = wp.tile([128, 256], BF16, tag="hq")
                    nc.scalar.square(hq[:], hs)
                    nc.tensor.matmul(
                        ps1[:], lhsT=onescol[:], rhs=hs,
                        start=(j == 0), stop=(j == gchunks - 1),
                    )
                    nc.tensor.matmul(
                        ps2[:], lhsT=onescol[:], rhs=hq[:],
                        start=(j == 0), stop=(j == gchunks - 1),
                    )
                nc.vector.tensor_add(acc_s[:, 0:256], acc_s[:, 0:256], ps1[:])
                nc.vector.tensor_add(acc_s[:, 256:512], acc_s[:, 256:512], ps2[:])

            # ---- AllReduce #2 (batchnorm sums) + affine constants
            nc.sync.dma_start(cc2_in[:], acc_s[:])
            nc.gpsimd.collective_compute(
                "AllReduce",
                mybir.AluOpType.add,
                replica_groups=[list(range(NCORES))],
                ins=[cc2_in[:]],
                outs=[cc2_out[:]],
            )
            gsd = cp.tile([1, 512], F32, tag="gsd")
            nc.sync.dma_start(gsd[:], cc2_out[:])

            mu = cp.tile([1, 256], F32, tag="mu")
            nc.scalar.mul(mu[:], gsd[:, 0:256], INV_N)
            ex2 = cp.tile([1, 256], F32, tag="ex2")
            nc.scalar.mul(ex2[:], gsd[:, 256:512], INV_N)
            mu2 = cp.tile([1, 256], F32, tag="mu2")
            nc.vector.tensor_mul(mu2[:], mu[:], mu[:])
            var = cp.tile([1, 256], F32, tag="var")
            nc.vector.tensor_sub(var[:], ex2[:], mu2[:])
            veps = cp.tile([1, 1], F32, tag="veps")
            nc.vector.memset(veps[:], EPS)
            std = cp.tile([1, 256], F32, tag="std")
            nc.scalar.activation(
                std[:], var[:], mybir.ActivationFunctionType.Sqrt, bias=veps[:]
            )
            rstd = cp.tile([1, 256], F32, tag="rstd")
            nc.vector.reciprocal(rstd[:], std[:])
            ab = cp.tile([1, 512], F32, tag="ab")
            nc.vector.tensor_mul(ab[:, 0:256], rstd[:], gb_sb[:, 0:256])
            mua = cp.tile([1, 256], F32, tag="mua")
            nc.vector.tensor_mul(mua[:], mu[:], ab[:, 0:256])
            nc.vector.tensor_sub(ab[:, 256:512], gb_sb[:, 256:512], mua[:])

            psb = psS.tile([128, 512], F32, tag="psb")
            nc.tensor.matmul(psb[:], lhsT=ones1_sb[:], rhs=ab[:], start=True, stop=True)
            A_b = cp.tile([128, 256], BF16, tag="A_b")
            nc.vector.tensor_copy(A_b[:], psb[:, 0:256])
            B_b = cp.tile([128, 256], F32, tag="B_b")
            nc.vector.tensor_copy(B_b[:], psb[:, 256:512])

            # ---- Stage E: out = relu(h * A + B + x)
            for t in range(T):
                xt = xp.tile([128, 256], F32, tag="xt")
                nc.sync.dma_start(xt[:], xs[t * 128 : (t + 1) * 128, :])
                xb2 = wp.tile([128, 256], F32, tag="xb2")
                nc.gpsimd.tensor_add(xb2[:], xt[:], B_b[:])
                m = wp.tile([128, 256], F32, tag="m")
                nc.vector.tensor_mul(m[:], hbuf[:, t, :], A_b[:])
                s = wp.tile([128, 256], F32, tag="s")
                nc.vector.tensor_add(s[:], m[:], xb2[:])
                ot = op_.tile([128, 256], F32, tag="ot")
                nc.scalar.activation(ot[:], s[:], mybir.ActivationFunctionType.Relu)
                nc.sync.dma_start(out_d[t * 128 : (t + 1) * 128, :], ot[:])

            psS.release()
            psH.release()

    nc.compile()
    return nc


def _get_program():
    global _CACHED_PROGRAM
    if _CACHED_PROGRAM is None:
        _CACHED_PROGRAM = _build_program()
    return _CACHED_PROGRAM


def _plan_core(x_s, d_s, a_s, e_s, cfg=None):
    """Bucket one core's rows by atom window; return padded arrays + row map."""
    cfg = cfg or Cfg()
    TROWS, BUCKET, RPC = cfg.trows, cfg.bucket, cfg.rpc
    bucket = (a_s >> 7).astype(np.int64)
    order = np.argsort(bucket, kind="stable")
    counts = np.bincount(bucket, minlength=NW)
    if counts.max() > BUCKET:
        raise RuntimeError(f"window overflow: {counts.max()} > {BUCKET}")

    xp_ = np.zeros((TROWS, 2 * NAE), np.float32)
    dp_ = np.zeros((TROWS, NDE), np.float32)
    awp = np.full((TROWS, 1), -1.0, np.float32)
    ewp = np.full((TROWS, 1), -1.0, np.float32)
    agp = np.full(TROWS, ZROW_A, np.int16)
    egp = np.full(TROWS, ZROW_E, np.int16)
    # device row position of each original row
    pos = np.empty(RPC, np.int64)

    start = 0
    for w in range(NW):
        k = counts[w]
        rows = order[start : start + k]
        start += k
        b = w * BUCKET
        xp_[b : b + k] = x_s[rows]
        dp_[b : b + k] = d_s[rows]
        awp[b : b + k, 0] = (a_s[rows] - 128 * w).astype(np.float32)
        ewp[b : b + k, 0] = e_s[rows].astype(np.float32)
        agp[b : b + k] = a_s[rows].astype(np.int16)
        egp[b : b + k] = e_s[rows].astype(np.int16)
        pos[rows] = np.arange(b, b + k)

    dsTp = np.ascontiguousarray(dp_.T)
    ag16 = np.ascontiguousarray(np.tile(agp.reshape(-1, 16).T, (8, 1)))
    eg16 = np.ascontiguousarray(np.tile(egp.reshape(-1, 16).T, (8, 1)))
    return xp_, dsTp, awp, ewp, ag16, eg16, pos


def _prepare(x, dist_feat, atom_idx, ele_idx, W1, gamma, beta, cfg=None):
    """Shard+plan all cores; returns (in_maps, positions)."""
    cfg = cfg or Cfg()
    x = np.ascontiguousarray(np.asarray(x, dtype=np.float32))
    dist_feat = np.ascontiguousarray(np.asarray(dist_feat, dtype=np.float32))
    atom_idx = np.asarray(atom_idx).astype(np.int64)
    ele_idx = np.asarray(ele_idx).astype(np.int64)
    W1 = np.ascontiguousarray(np.asarray(W1, dtype=np.float32))
    gamma = np.asarray(gamma, dtype=np.float32)
    beta = np.asarray(beta, dtype=np.float32)

    rc = np.zeros((SUMW,), np.float32)
    rc[:G] = 1.0 / np.maximum(np.bincount(atom_idx, minlength=G), 1.0)
    rc[G : G + E] = 1.0 / np.maximum(np.bincount(ele_idx, minlength=E), 1.0)
    rcb = np.ascontiguousarray(np.broadcast_to(rc, (128, SUMW))).astype(np.float32)
    arange = np.ascontiguousarray(
        np.broadcast_to(np.arange(128, dtype=np.float32), (128, 128))
    )
    ones1 = np.ones((1, 128), np.float32)
    gb = np.concatenate([gamma, beta]).reshape(1, 512).astype(np.float32)

    in_maps = []
    positions = []
    for c in range(NCORES):
        sl = slice(c * cfg.rpc, (c + 1) * cfg.rpc)
        xp_, dsTp, awp, ewp, ag16, eg16, pos = _plan_core(
            x[sl], dist_feat[sl], atom_idx[sl], ele_idx[sl], cfg
        )
        positions.append(pos)
        in_maps.append(
            {
                "xs": xp_,
                "dsT": dsTp,
                "aw": awp,
                "ew": ewp,
                "ag16": ag16,
                "eg16": eg16,
                "w1": W1,
                "gb": gb,
                "rcb": rcb,
                "arange": arange,
                "ones1": ones1,
            }
        )
    return in_maps, positions


def kernel(x, dist_feat, atom_idx, ele_idx, W1, b1, gamma, beta, num_graphs, num_eles):
    assert int(num_graphs) == G and int(num_eles) == E
    assert np.asarray(x).shape == (N, 2 * NAE)

    nc = _get_program()
    in_maps, positions = _prepare(x, dist_feat, atom_idx, ele_idx, W1, gamma, beta)
    res = run_bass_kernel_spmd(nc, in_maps, core_ids=list(range(NCORES)))

    out = np.empty((N, 2 * NAE), np.float32)
    for c in range(NCORES):
        dev = res.results[c]["out"]
        out[c * RPC : (c + 1) * RPC] = dev[positions[c]]
    return out


# revision 29
# speedup vs baseline: 2.6168x; 2.6168x over previous
"""Trainium2 Bass kernel for nn_DistLayer (GNN message passing layer).

Computes, for full inputs (see reference):
    pa = relu(seg_mean(x[:, :128], atom_idx, 1024))[atom_idx]
    pe = relu(seg_mean(x[:, 128:], ele_idx, 100))[ele_idx]
    h  = concat([dist_feat, pa, pe], 1) @ W1 (+ b1)
    out = relu(batchnorm_train(h; gamma, beta) + x)

Note b1 provably cancels in (h - mean(h)), so it is ignored.

Strategy (8 cores, data-parallel over rows):
  - Rows sharded 25000/core; each shard bucketed by atom_idx>>7 into 8
    fixed-size 3456-row windows (pad rows are inert), so segment sums and
    the gather-back both use narrow [128,128] one-hot matmuls.
  - AllReduce #1 combines per-core segment sums [128, 1152].
  - Pooled means -> relu -> matmul with W1 halves gives per-segment
    contribution tables kept in SBUF; rows are expanded back with
    transposed one-hot matmuls accumulated straight into the h PSUM.
  - h kept resident in SBUF (bf16); BN stats via ones-matmul column sums,
    AllReduce #2, then fused affine+residual+relu output pass.
"""
import sys

sys.path.insert(0, "/opt/trn_rl_repo")

import numpy as np

import concourse.bass as bass
import concourse.mybir as mybir
import concourse.tile as tile
from concourse import bacc
from concourse.bass_utils import run_bass_kernel_spmd, axon_active

# problem constants
N = 200000
NAE = 128
NDE = 128
G = 1024
E = 100
NCORES = 8
RPC = N // NCORES          # 25000 rows per core
NW = 8                     # windows (atom segment buckets of 128)
CPW = 27                   # chunks (of 128 rows) per window
BUCKET = CPW * 128         # 3456 padded rows per window
TROWS = NW * BUCKET        # 27648 padded rows per core
T = TROWS // 128           # 216 chunks
SUMW = G + 128             # 1152: [atom sums | ele sums(padded to 128)]
EPS = 1e-5
INV_N = 1.0 / N

F32 = mybir.dt.float32
BF16 = mybir.dt.bfloat16

_CACHED_PROGRAM = None


class Cfg:
    """Size configuration; defaults = the real problem."""

    def __init__(self, rpc=RPC, cpw=CPW, gg=None, debug=None):
        self.rpc = rpc
        self.cpw = cpw
        self.bucket = cpw * 128
        self.trows = NW * self.bucket
        self.t = self.trows // 128
        assert self.t % 8 == 0
        self.inv_n = 1.0 / (rpc * NCORES)
        self.debug = debug


def _build_program(cfg=None):
    cfg = cfg or Cfg()
    CPW, TROWS, T = cfg.cpw, cfg.trows, cfg.t
    INV_N = cfg.inv_n
    dbg = (not axon_active()) if cfg.debug is None else cfg.debug
    nc = bacc.Bacc(
        "TRN2",
        target_bir_lowering=False,
        debug=dbg,
        num_devices=NCORES,
    )

    # per-core external I/O (all activations pre-converted to bf16 on host)
    xsb = nc.dram_tensor("xsb", [TROWS, 2 * NAE], BF16, kind="ExternalInput")
    dsTb = nc.dram_tensor("dsTb", [NDE, TROWS], BF16, kind="ExternalInput")
    ohra = nc.dram_tensor("ohra", [TROWS, 128], BF16, kind="ExternalInput")
    ohre = nc.dram_tensor("ohre", [TROWS, 128], BF16, kind="ExternalInput")
    ohta = nc.dram_tensor("ohta", [128, TROWS], BF16, kind="ExternalInput")
    ohte = nc.dram_tensor("ohte", [128, TROWS], BF16, kind="ExternalInput")
    w1 = nc.dram_tensor("w1", [3 * 128, 2 * NAE], F32, kind="ExternalInput")
    gb = nc.dram_tensor("gb", [1, 512], F32, kind="ExternalInput")
    rcb = nc.dram_tensor("rcb", [128, SUMW], F32, kind="ExternalInput")
    ones1 = nc.dram_tensor("ones1", [1, 128], F32, kind="ExternalInput")
    out_d = nc.dram_tensor("out", [TROWS, 2 * NAE], F32, kind="ExternalOutput")

    # internal DRAM (collective bounce buffers)
    cc1_in = nc.dram_tensor("cc1_in", [128, SUMW], F32)
    cc1_out = nc.dram_tensor("cc1_out", [128, SUMW], F32, addr_space="Shared")
    cc2_in = nc.dram_tensor("cc2_in", [1, 1024], F32)
    cc2_out = nc.dram_tensor("cc2_out", [1, 1024], F32, addr_space="Shared")

    RELU = mybir.ActivationFunctionType.Relu
    SQUARE = mybir.ActivationFunctionType.Square
    SQRT = mybir.ActivationFunctionType.Sqrt
    ISEQ = mybir.AluOpType.is_equal

    NQ = T // 4                      # quads (4-chunk groups)
    FG = 9 if NQ % 9 == 0 else NQ    # sums flush-group size in quads

    with tile.TileContext(nc) as tc:
        with (
            tc.tile_pool(name="const", bufs=1) as cp,
            tc.tile_pool(name="hcache", bufs=1) as hp,
            tc.tile_pool(name="xload", bufs=2) as xp,
            tc.tile_pool(name="dload", bufs=2) as dp,
            tc.tile_pool(name="work", bufs=2) as wp,
            tc.tile_pool(name="outp", bufs=2) as op_,
        ):
            # ---- constants into SBUF
            w1bf = []
            for i in range(3):
                tf = wp.tile([128, 256], F32, tag="w1f")
                nc.sync.dma_start(tf[:], w1[i * 128 : (i + 1) * 128, :])
                tb = cp.tile([128, 256], BF16, tag=f"w1b{i}")
                nc.scalar.copy(tb[:], tf[:])
                w1bf.append(tb)
            w1d, w1a, w1e = w1bf

            rcb_sb = cp.tile([128, SUMW], F32, tag="rcb")
            nc.sync.dma_start(rcb_sb[:], rcb[:])
            ones1_sb = cp.tile([1, 128], F32, tag="ones1")
            nc.sync.dma_start(ones1_sb[:], ones1[:])
            gb_sb = cp.tile([1, 512], F32, tag="gb")
            nc.sync.dma_start(gb_sb[:], gb[:])
            onescol = cp.tile([128, 1], BF16, tag="onescol")
            nc.vector.memset(onescol[:], 1.0)

            # ---- Stage A: local segment sums (transposed: [ae_dim, seg])
            acc = cp.tile([128, SUMW], F32, tag="acc")

            psA = tc.alloc_tile_pool(name="psA", bufs=2, space="PSUM")
            for w in range(NW):
                ps_a = psA.tile([128, 128], F32, tag="ps_a")
                ps_e = psA.tile([128, 128], F32, tag="ps_e")
                done = 0
                while done < CPW:
                    b = min(8, CPW - done)
                    t0 = w * CPW + done
                    rows = slice(t0 * 128, (t0 + b) * 128)
                    xq = xp.tile([128, 8, 256], BF16, tag="xq")
                    nc.sync.dma_start(
                        xq[:, 0:b, :],
                        xsb[rows, :].rearrange("(n p) m -> p n m", p=128),
                    )
                    ra = wp.tile([128, 8, 128], BF16, tag="ra")
                    nc.sync.dma_start(
                        ra[:, 0:b, :],
                        ohra[rows, :].rearrange("(n p) m -> p n m", p=128),
                    )
                    re = wp.tile([128, 8, 128], BF16, tag="re")
                    nc.sync.dma_start(
                        re[:, 0:b, :],
                        ohre[rows, :].rearrange("(n p) m -> p n m", p=128),
                    )
                    for j in range(b):
                        nc.tensor.matmul(
                            ps_a[:], lhsT=xq[:, j, 0:128], rhs=ra[:, j, :],
                            start=(done + j == 0), stop=(done + j == CPW - 1),
                        )
                        nc.tensor.matmul(
                            ps_e[:], lhsT=xq[:, j, 128:256], rhs=re[:, j, :],
                            start=(done + j == 0), stop=(done + j == CPW - 1),
                        )
                    done += b
                nc.vector.tensor_copy(acc[:, w * 128 : (w + 1) * 128], ps_a[:])
                if w == 0:
                    nc.vector.tensor_copy(acc[:, G : G + 128], ps_e[:])
                else:
                    nc.vector.tensor_add(
                        acc[:, G : G + 128], acc[:, G : G + 128], ps_e[:]
                    )
            psA.release()
            psH = tc.alloc_tile_pool(name="psH", bufs=2, space="PSUM")
            psS = tc.alloc_tile_pool(name="psS", bufs=1, space="PSUM")

            # ---- AllReduce #1 (segment sums)
            nc.sync.dma_start(cc1_in[:], acc[:])
            nc.gpsimd.collective_compute(
                "AllReduce",
                mybir.AluOpType.add,
                replica_groups=[list(range(NCORES))],
                ins=[cc1_in[:]],
                outs=[cc1_out[:]],
            )
            nc.sync.dma_start(acc[:], cc1_out[:])

            # ---- tables: relu(mean) @ W1 part, kept in SBUF (bf16)
            nc.vector.tensor_mul(acc[:], acc[:], rcb_sb[:])
            rmeans = cp.tile([128, SUMW], BF16, tag="rmeans")
            nc.scalar.activation(rmeans[:], acc[:], RELU)

            tbl_a = cp.tile([128, NW, 256], BF16, tag="tbl_a")
            for blk in range(NW):
                pst = psH.tile([128, 512], F32, tag="psbc")
                nc.tensor.matmul(
                    pst[:, 0:256],
                    lhsT=rmeans[:, blk * 128 : (blk + 1) * 128],
                    rhs=w1a[:],
                    start=True,
                    stop=True,
                )
                nc.scalar.copy(tbl_a[:, blk, :], pst[:, 0:256])
            tbl_e = cp.tile([128, 256], BF16, tag="tbl_e")
            pst = psH.tile([128, 512], F32, tag="psbc")
            nc.tensor.matmul(
                pst[:, 0:256], lhsT=rmeans[:, G : G + 128], rhs=w1e[:],
                start=True, stop=True,
            )
            nc.scalar.copy(tbl_e[:], pst[:, 0:256])

            # ---- Stage C: h = dsT.T@W1d + onehotT_a.T@tbl_a + onehotT_e.T@tbl_e
            hbuf = hp.tile([128, T, 256], BF16, tag="H")
            acc_s1 = cp.tile([1, 512], F32, tag="acc_s1")
            acc_s2 = cp.tile([1, 512], F32, tag="acc_s2")

            ps1 = ps2 = None
            dq = oa = oe = None
            for q in range(NQ):
                if q % 2 == 0:
                    cols = slice(q * 512, (q + 2) * 512)
                    dq = dp.tile([128, 1024], BF16, tag="dq")
                    nc.sync.dma_start(dq[:, 0 : min(1024, TROWS - q * 512)],
                                      dsTb[:, cols])
                    oa = dp.tile([128, 1024], BF16, tag="oa")
                    nc.sync.dma_start(oa[:, 0 : min(1024, TROWS - q * 512)],
                                      ohta[:, cols])
                    oe = dp.tile([128, 1024], BF16, tag="oe")
                    nc.sync.dma_start(oe[:, 0 : min(1024, TROWS - q * 512)],
                                      ohte[:, cols])
                off = (q % 2) * 512
                ps4 = psH.tile([128, 4, 256], F32, tag="ps4")
                for k in range(4):
                    t = q * 4 + k
                    w = t // CPW
                    sl = slice(off + k * 128, off + (k + 1) * 128)
                    nc.tensor.matmul(
                        ps4[:, k, :], lhsT=dq[:, sl], rhs=w1d[:],
                        start=True, stop=False,
                    )
                    nc.tensor.matmul(
                        ps4[:, k, :], lhsT=oa[:, sl], rhs=tbl_a[:, w, :],
                        start=False, stop=False,
                    )
                    nc.tensor.matmul(
                        ps4[:, k, :], lhsT=oe[:, sl], rhs=tbl_e[:],
                        start=False, stop=True,
                    )
                hs = hbuf[:, q * 4 : (q + 1) * 4, :]
                nc.scalar.copy(hs, ps4[:])
                hq = wp.tile([128, 4, 256], BF16, tag="hq")
                nc.scalar.activation(hq[:], hs, SQUARE)

                gfirst = q % FG == 0
                glast = q % FG == FG - 1 or q == NQ - 1
                if gfirst:
                    ps1 = psS.tile([1, 512], F32, tag="ps1")
                    ps2 = psS.tile([1, 512], F32, tag="ps2")
                for hf in range(2):
                    sl2 = hbuf[:, q * 4 + 2 * hf : q * 4 + 2 * hf + 2, :]
                    nc.tensor.matmul(
                        ps1[:], lhsT=onescol[:],
                        rhs=sl2.rearrange("p n m -> p (n m)"),
                        start=(gfirst and hf == 0), stop=(glast and hf == 1),
                    )
                    nc.tensor.matmul(
                        ps2[:], lhsT=onescol[:],
                        rhs=hq[:, 2 * hf : 2 * hf + 2, :].rearrange(
                            "p n m -> p (n m)"
                        ),
                        start=(gfirst and hf == 0), stop=(glast and hf == 1),
                    )
                if glast:
                    if q < FG:
                        nc.vector.tensor_copy(acc_s1[:], ps1[:])
                        nc.vector.tensor_copy(acc_s2[:], ps2[:])
                    else:
                        nc.vector.tensor_add(acc_s1[:], acc_s1[:], ps1[:])
                        nc.vector.tensor_add(acc_s2[:], acc_s2[:], ps2[:])

            # ---- AllReduce #2 (batchnorm sums) + affine constants
            sdt = cp.tile([1, 1024], F32, tag="sdt")
            nc.vector.tensor_copy(sdt[:, 0:512], acc_s1[:])
            nc.vector.tensor_copy(sdt[:, 512:1024], acc_s2[:])
            nc.sync.dma_start(cc2_in[:], sdt[:])
            nc.gpsimd.collective_compute(
                "AllReduce",
                mybir.AluOpType.add,
                replica_groups=[list(range(NCORES))],
                ins=[cc2_in[:]],
                outs=[cc2_out[:]],
            )
            nc.sync.dma_start(sdt[:], cc2_out[:])

            s1f = cp.tile([1, 256], F32, tag="s1f")
            nc.vector.tensor_add(s1f[:], sdt[:, 0:256], sdt[:, 256:512])
            s2f = cp.tile([1, 256], F32, tag="s2f")
            nc.vector.tensor_add(s2f[:], sdt[:, 512:768], sdt[:, 768:1024])
            mu = cp.tile([1, 256], F32, tag="mu")
            nc.scalar.mul(mu[:], s1f[:], INV_N)
            ex2 = cp.tile([1, 256], F32, tag="ex2")
            nc.scalar.mul(ex2[:], s2f[:], INV_N)
            mu2 = cp.tile([1, 256], F32, tag="mu2")
            nc.vector.tensor_mul(mu2[:], mu[:], mu[:])
            var = cp.tile([1, 256], F32, tag="var")
            nc.vector.tensor_sub(var[:], ex2[:], mu2[:])
            veps = cp.tile([1, 1], F32, tag="veps")
            nc.vector.memset(veps[:], EPS)
            std = cp.tile([1, 256], F32, tag="std")
            nc.scalar.activation(std[:], var[:], SQRT, bias=veps[:])
            rstd = cp.tile([1, 256], F32, tag="rstd")
            nc.vector.reciprocal(rstd[:], std[:])
            ab = cp.tile([1, 512], F32, tag="ab")
            nc.vector.tensor_mul(ab[:, 0:256], rstd[:], gb_sb[:, 0:256])
            mua = cp.tile([1, 256], F32, tag="mua")
            nc.vector.tensor_mul(mua[:], mu[:], ab[:, 0:256])
            nc.vector.tensor_sub(ab[:, 256:512], gb_sb[:, 256:512], mua[:])

            psb = psH.tile([128, 512], F32, tag="psbc")
            nc.tensor.matmul(
                psb[:], lhsT=ones1_sb[:], rhs=ab[:], start=True, stop=True
            )
            A_b8 = cp.tile([128, 8, 256], BF16, tag="A_b8")
            B_b8 = cp.tile([128, 8, 256], BF16, tag="B_b8")
            for j in range(8):
                nc.scalar.copy(A_b8[:, j, :], psb[:, 0:256])
                nc.scalar.copy(B_b8[:, j, :], psb[:, 256:512])

            # ---- Stage E: out = relu(h * A + B + x)
            NO = T // 8
            for o in range(NO):
                rows = slice(o * 1024, (o + 1) * 1024)
                xb8 = xp.tile([128, 8, 256], BF16, tag="xq")
                nc.sync.dma_start(
                    xb8[:], xsb[rows, :].rearrange("(n p) m -> p n m", p=128)
                )
                m8 = wp.tile([128, 8, 256], BF16, tag="m8")
                nc.vector.tensor_mul(
                    m8[:], hbuf[:, o * 8 : (o + 1) * 8, :], A_b8[:]
                )
                nc.vector.tensor_add(m8[:], m8[:], xb8[:])
                nc.vector.tensor_add(m8[:], m8[:], B_b8[:])
                for h2 in range(2):
                    ot = op_.tile([128, 4, 256], F32, tag="ot")
                    nc.scalar.activation(ot[:], m8[:, h2 * 4 : (h2 + 1) * 4, :], RELU)
                    r2 = slice(o * 1024 + h2 * 512, o * 1024 + (h2 + 1) * 512)
                    nc.sync.dma_start(
                        out_d[r2, :].rearrange("(n p) m -> p n m", p=128),
                        ot[:],
                    )

            psS.release()
            psH.release()

    nc.compile()
    return nc


def _get_program():
    global _CACHED_PROGRAM
    if _CACHED_PROGRAM is None:
        _CACHED_PROGRAM = _build_program()
    return _CACHED_PROGRAM


def _plan_core(x_s, d_s, a_s, e_s, cfg=None):
    """Bucket one core's rows by atom window; return padded arrays + row map."""
    import ml_dtypes

    cfg = cfg or Cfg()
    TROWS, BUCKET, RPC, T = cfg.trows, cfg.bucket, cfg.rpc, cfg.t
    bucket = (a_s >> 7).astype(np.int64)
    order = np.argsort(bucket, kind="stable")
    counts = np.bincount(bucket, minlength=NW)
    if counts.max() > BUCKET:
        raise RuntimeError(f"window overflow: {counts.max()} > {BUCKET}")

    BF = ml_dtypes.bfloat16
    xp_ = np.zeros((TROWS, 2 * NAE), BF)
    dp_ = np.zeros((TROWS, NDE), np.float32)
    awp = np.full(TROWS, -1, np.int64)
    ewp = np.full(TROWS, -1, np.int64)
    pos = np.empty(RPC, np.int64)

    start = 0
    for w in range(NW):
        k = counts[w]
        rows = order[start : start + k]
        start += k
        b = w * BUCKET
        xp_[b : b + k] = x_s[rows].astype(BF)
        dp_[b : b + k] = d_s[rows]
        awp[b : b + k] = a_s[rows] - 128 * w
        ewp[b : b + k] = e_s[rows]
        pos[rows] = np.arange(b, b + k)

    dsTb = np.ascontiguousarray(dp_.T).astype(BF)
    ar = np.arange(128, dtype=np.int64)
    ohra = (awp[:, None] == ar[None, :]).astype(BF)
    ohre = (ewp[:, None] == ar[None, :]).astype(BF)
    ohta = np.ascontiguousarray(ohra.T)
    ohte = np.ascontiguousarray(ohre.T)
    return xp_, dsTb, ohra, ohre, ohta, ohte, pos


def _prepare(x, dist_feat, atom_idx, ele_idx, W1, gamma, beta, cfg=None):
    """Shard+plan all cores; returns (in_maps, positions)."""
    cfg = cfg or Cfg()
    x = np.ascontiguousarray(np.asarray(x, dtype=np.float32))
    dist_feat = np.ascontiguousarray(np.asarray(dist_feat, dtype=np.float32))
    atom_idx = np.asarray(atom_idx).astype(np.int64)
    ele_idx = np.asarray(ele_idx).astype(np.int64)
    W1 = np.ascontiguousarray(np.asarray(W1, dtype=np.float32))
    gamma = np.asarray(gamma, dtype=np.float32)
    beta = np.asarray(beta, dtype=np.float32)

    rc = np.zeros((SUMW,), np.float32)
    rc[:G] = 1.0 / np.maximum(np.bincount(atom_idx, minlength=G), 1.0)
    rc[G : G + E] = 1.0 / np.maximum(np.bincount(ele_idx, minlength=E), 1.0)
    rcb = np.ascontiguousarray(np.broadcast_to(rc, (128, SUMW))).astype(np.float32)
    ones1 = np.ones((1, 128), np.float32)
    gbv = np.concatenate([gamma, beta]).reshape(1, 512).astype(np.float32)

    in_maps = []
    positions = []
    for c in range(NCORES):
        sl = slice(c * cfg.rpc, (c + 1) * cfg.rpc)
        xsb, dsTb, ohra, ohre, ohta, ohte, pos = _plan_core(
            x[sl], dist_feat[sl], atom_idx[sl], ele_idx[sl], cfg
        )
        positions.append(pos)
        in_maps.append(
            {
                "xsb": xsb,
                "dsTb": dsTb,
                "ohra": ohra,
                "ohre": ohre,
                "ohta": ohta,
                "ohte": ohte,
                "w1": W1,
                "gb": gbv,
                "rcb": rcb,
                "ones1": ones1,
            }
        )
    return in_maps, positions


def kernel(x, dist_feat, atom_idx, ele_idx, W1, b1, gamma, beta, num_graphs, num_eles):
    assert int(num_graphs) == G and int(num_eles) == E
    assert np.asarray(x).shape == (N, 2 * NAE)

    nc = _get_program()
    in_maps, positions = _prepare(x, dist_feat, atom_idx, ele_idx, W1, gamma, beta)
    res = run_bass_kernel_spmd(nc, in_maps, core_ids=list(range(NCORES)))

    out = np.empty((N, 2 * NAE), np.float32)
    for c in range(NCORES):
        dev = res.results[c]["out"]
        out[c * RPC : (c + 1) * RPC] = dev[positions[c]]
    return out
